# revision 27
# baseline (speedup 1.0000x reference)
"""Capsule dynamic-routing kernel for 8 TRN2 NeuronCores.

Problem: nn_CapsuleRouting — p:(16,32,16,14,14), W_ij:(32,4,4,32), 3 routing
iterations, returns (v:(16,32,16,14,14), a:(16,32,14,14)).

Sharding: data-parallel over batch (2 batch elems per core, 8 cores).

Per-core design:
  - 392 positions (2 batch x 196 hw) in 4 chunks of 98 partitions.
  - SBUF layout: partitions = positions; u votes stored [98, B=32, ik=16, C=32]
    bf16.  Softmax coeffs c[B,C,pos] and squashed votes v[C,ik,pos] broadcast
    along free dims with step-0 APs (no replication needed).
  - Vote einsum u = p4 @ W on PE: per (B,i) matmul, K=j=4, out [pos98,(k,C)].
  - Reductions over B (weighted sum -> s) and over ik (r update) via PSUM
    accumulation with identity-lhsT matmuls on PE, pipelined behind the DVE
    bf16 2x multiply passes (w = c*u or u*v).
  - iter0: c uniform -> s0 = (sum_B u)/32 via PE accumulation directly on u.
  - Small math (softmax exp, squash, reciprocal) on ACT/DVE, batched across
    all 4 chunks in single wide ops.
"""

import numpy as np
from contextlib import ExitStack

P = 4
PP = 16
B = 32
C = 32
H = 14
WW = 14
POS = H * WW          # 196
HALF = POS // 2       # 98
BLOC = 2              # batch elems per core
NCH = 4               # chunks per core: (batch, half)
ITERS = 3
EPS = 1e-5
NCORES = 8


def _build(debug=False):
    import concourse.bass as bass
    import concourse.bacc as bacc
    import concourse.tile as tile
    from concourse import mybir

    f32 = mybir.dt.float32
    bf16 = mybir.dt.bfloat16
    AX = mybir.AxisListType
    AF = mybir.ActivationFunctionType

    nc = bacc.Bacc()
    p_ext = nc.declare_dram_parameter("p", [BLOC, B, PP, H, WW], bf16, isOutput=False)
    wbd_ext = nc.declare_dram_parameter("Wbd", [128, B * 128], bf16, isOutput=False)
    wsum_ext = nc.declare_dram_parameter("Wsum", [128, 128], bf16, isOutput=False)
    id_ext = nc.declare_dram_parameter("ident", [128, 128], f32, isOutput=False)
    v_ext = nc.declare_dram_parameter("v_out", [BLOC, C, PP, POS], f32, isOutput=True)
    a_ext = nc.declare_dram_parameter("a_out", [BLOC, C, POS], f32, isOutput=True)
    if debug:
        du_ext = nc.declare_dram_parameter(
            "dbg_u", [NCH, 98, B, PP, C], bf16, isOutput=True)
        ds_ext = nc.declare_dram_parameter(
            "dbg_s", [98, NCH, PP, C], f32, isOutput=True)

    def bcast(sl, axis, count):
        """Insert a step-0 (broadcast) dim into AP `sl` at free-dim position
        `axis` (0 = first free dim)."""
        ap = list(sl.ap)
        ap.insert(1 + axis, [0, count])
        return bass.AP(tensor=sl.tensor, offset=sl.offset, ap=ap)

    with tile.TileContext(nc) as tc, ExitStack() as ctx:
        singles = ctx.enter_context(tc.tile_pool(name="singles", bufs=1))
        upool = ctx.enter_context(tc.tile_pool(name="upool", bufs=4))
        w2pool = ctx.enter_context(tc.tile_pool(name="w2pool", bufs=2))
        small = ctx.enter_context(tc.tile_pool(name="small", bufs=1))
        outp = ctx.enter_context(tc.tile_pool(name="outp", bufs=2))
        mmps = ctx.enter_context(tc.tile_pool(name="mmps", bufs=2, space="PSUM"))
        sps = ctx.enter_context(tc.tile_pool(name="sps", bufs=2, space="PSUM"))
        tps = ctx.enter_context(tc.tile_pool(name="tps", bufs=2, space="PSUM"))

        # --- constants ---
        id_f = singles.tile([128, 128], f32)
        nc.sync.dma_start(out=id_f, in_=id_ext[:, :])
        id_b = singles.tile([128, 128], bf16)
        nc.scalar.copy(id_b, id_f)
        eps_t = singles.tile([98, 1], f32)
        nc.vector.memset(eps_t, EPS)
        one_t = singles.tile([98, 1], f32)
        nc.vector.memset(one_t, 1.0)

        # --- persistent state (batched over the 4 chunks in free dim) ---
        u = [upool.tile([98, B, PP, C], bf16, tag="u", name=f"u{i}")
             for i in range(NCH)]
        r_all = small.tile([98, NCH, B, C], f32)       # routing logits
        c_all = small.tile([98, NCH, B, C], bf16)      # softmax coeffs
        s_all = small.tile([98, NCH, PP, C], f32)      # s, then v (in-place)
        vbf_all = small.tile([98, NCH, PP, C], bf16)   # v in bf16
        n2_all = small.tile([98, NCH, C], f32)
        nrm_all = small.tile([98, NCH, C], f32)
        t1_all = small.tile([98, NCH, C], f32)
        rc_all = small.tile([98, NCH, C], f32)
        sc_all = small.tile([98, NCH, C], f32)
        d_all = small.tile([98, NCH, B], f32)
        rcd_all = small.tile([98, NCH, B], f32)
        a_all = small.tile([98, NCH, C], f32)

        # --- phase 1: vote einsum u[pos, B, (i,k), C] = sum_j p4[B,i,j,pos] W[B,j,k,C]
        # Block-diagonal Wbd[(B,j), (B',k,C)] = delta_BB' W[B,j,k,C] gives all B
        # in one K=128 matmul per (i, 512-col piece).  Wsum (plain stacked W)
        # contracts over (B,j) directly -> s0 = sum_B u (iter0 softmax uniform).
        with tc.tile_pool(name="ppool", bufs=2) as ppool, \
             tc.tile_pool(name="wpool", bufs=1) as wpool:
            wbd = wpool.tile([128, B * 128], bf16)
            nc.sync.dma_start(out=wbd, in_=wbd_ext[:, :])
            wsum = wpool.tile([128, 128], bf16)
            nc.sync.dma_start(out=wsum, in_=wsum_ext[:, :])
            for ch in range(NCH):
                bi, hf = divmod(ch, 2)
                src = p_ext[bi].rearrange("b (i j) h w -> b j i (h w)", i=P, j=P)
                p_b = ppool.tile([128, P, HALF], bf16, tag="pb")  # [(B,j), i, pos]
                for i in range(P):
                    nc.sync.dma_start(
                        out=p_b[:, i, :],
                        in_=src[:, :, i, hf * HALF:(hf + 1) * HALF],
                    )
                # s0 for iter0: one K=128 matmul per i, all four i in one bank
                s0 = mmps.tile([98, P, 128], f32, tag="mm")
                for i in range(P):
                    nc.tensor.matmul(
                        s0[:, i, :], p_b[:, i, :], wsum,
                        start=True, stop=True,
                    )
                nc.scalar.activation(
                    s_all[:, ch, :, :], s0.rearrange("q i n -> q (i n)"),
                    AF.Copy, scale=1.0 / B,
                )
                for i in range(P):
                    for hp in range(4):          # pairs of 512-col pieces
                        ps = mmps.tile([98, 8, P, C], f32, tag="mm")
                        for t in range(2):
                            pc = hp * 2 + t      # piece: B' in [pc*4, pc*4+4)
                            nc.tensor.matmul(
                                ps[:, t * 4:(t + 1) * 4, :, :], p_b[:, i, :],
                                wbd[:, pc * 512:(pc + 1) * 512],
                                start=True, stop=True,
                            )
                        nc.scalar.copy(
                            u[ch][:, hp * 8:(hp + 1) * 8, i * P:(i + 1) * P, :],
                            ps,
                        )

        def squash(it):
            """s_all -> v (in-place); sets n2/nrm/sc; uses w2pool for squares."""
            sq = w2pool.tile([98, NCH * PP * C], f32, tag="w2")
            nc.scalar.square(sq, s_all[:, :, :, :])
            sqv = sq.rearrange("q (n i c) -> q n i c", n=NCH, i=PP)
            # reduce over ik (strided view, C kept)
            nc.vector.tensor_reduce(
                n2_all[:, :, :],
                sqv.rearrange("q n i c -> q n c i"),
                axis=AX.X, op=mybir.AluOpType.add,
            )
            nc.scalar.activation(nrm_all, n2_all, AF.Sqrt, bias=eps_t)
            nc.scalar.activation(t1_all, n2_all, AF.Identity, bias=one_t)
            nc.vector.tensor_mul(t1_all, t1_all, nrm_all)      # (1+n2)*nrm
            nc.vector.reciprocal(rc_all, t1_all)
            nc.vector.tensor_mul(sc_all, rc_all, n2_all)       # n2/((1+n2)nrm)
            # v = s * scale (broadcast over ik), in-place on s_all
            nc.vector.tensor_mul(
                s_all[:, :, :, :],
                s_all[:, :, :, :],
                bcast(sc_all[:, :, :], 1, PP),
            )
            if it != ITERS - 1:
                nc.scalar.copy(vbf_all, s_all[:, :, :, :])

        def softmax():
            nc.scalar.activation(c_all[:, :, :, :], r_all[:, :, :, :], AF.Exp)
            nc.vector.tensor_reduce(
                d_all[:, :, :], c_all[:, :, :, :],
                axis=AX.X, op=mybir.AluOpType.add,
            )
            nc.vector.reciprocal(rcd_all, d_all)
            nc.vector.tensor_mul(
                c_all[:, :, :, :],
                c_all[:, :, :, :],
                bcast(rcd_all[:, :, :], 2, C),
            )

        if debug:
            for ch in range(NCH):
                nc.sync.dma_start(out=du_ext[ch], in_=u[ch])
            nc.sync.dma_start(out=ds_ext[:, :, :, :], in_=s_all)

        # --- routing iterations (s_all for iter0 was filled by the einsum) ---
        for it in range(ITERS):
            if it > 0:
                # s-pass: s[pos,(ik,C)] = sum_B c*u
                for ch in range(NCH):
                    sp = sps.tile([98, PP, C], f32, tag="sp")
                    for g in range(4):            # 8 B-values per sub
                        w2 = w2pool.tile([98, 8, PP, C], bf16, tag="w2")
                        csl = c_all[:, ch, g * 8:(g + 1) * 8, :]
                        nc.vector.tensor_mul(
                            w2, u[ch][:, g * 8:(g + 1) * 8, :, :],
                            bcast(csl, 1, PP),
                        )
                        for b8 in range(8):
                            nc.tensor.matmul(
                                sp[:, :, :], id_b[:98, :98], w2[:, b8, :, :],
                                start=(g == 0 and b8 == 0),
                                stop=(g == 3 and b8 == 7),
                            )
                    nc.scalar.copy(s_all[:, ch, :, :], sp)
            squash(it)
            if it == ITERS - 1:
                break
            # dr-pass: dr[pos,(B,C)] = sum_ik u*v ; r += dr
            for ch in range(NCH):
                dp = mmps.tile([98, B, C], f32, tag="mm")
                for g in range(4):
                    w2 = w2pool.tile([98, 8, PP, C], bf16, tag="w2")
                    vsl = vbf_all[:, ch, :, :]
                    nc.vector.tensor_mul(
                        w2, u[ch][:, g * 8:(g + 1) * 8, :, :],
                        bcast(vsl, 0, 8),
                    )
                    for ik in range(PP):
                        nc.tensor.matmul(
                            dp[:, g * 8:(g + 1) * 8, :],
                            id_b[:98, :98],
                            w2[:, :, ik, :],
                            start=(ik == 0), stop=(ik == PP - 1),
                        )
                if it == 0:
                    nc.scalar.copy(r_all[:, ch, :, :], dp)
                else:
                    nc.vector.tensor_add(r_all[:, ch, :, :], r_all[:, ch, :, :], dp)
            softmax()

        # a = sqrt((n2/(1+n2))^2 + eps) ;  n2/(1+n2) = sc*nrm
        nc.vector.tensor_mul(a_all, sc_all, nrm_all)
        nc.scalar.square(a_all, a_all)
        nc.scalar.activation(a_all, a_all, AF.Sqrt, bias=eps_t)

        # --- outputs: transpose [98, x] -> [x, 98] on PE, then DMA ---
        for ch in range(NCH):
            bi, hf = divmod(ch, 2)
            vdst = v_ext[bi].rearrange("c (i k) s -> k c i s", i=P, k=P)
            for q in range(P):      # i = q slice of (ik,C) flat
                tp = tps.tile([128, 98], f32, tag="tp")
                nc.tensor.transpose(
                    tp, s_all[:, ch, q * P:(q + 1) * P, :].rearrange("q a b -> q (a b)"),
                    id_f[:98, :98],
                )
                vt = outp.tile([128, 98], f32, tag="vt")
                nc.scalar.copy(vt, tp)
                nc.sync.dma_start(
                    out=vdst[:, :, q, hf * HALF:(hf + 1) * HALF],
                    in_=vt,
                )
            tp = tps.tile([128, 98], f32, tag="tp")
            nc.tensor.transpose(tp[:C, :], a_all[:, ch, :], id_f[:98, :98])
            at = outp.tile([C, 98], f32, tag="at")
            nc.scalar.copy(at, tp[:C, :])
            nc.sync.dma_start(
                out=a_ext[bi, :, hf * HALF:(hf + 1) * HALF], in_=at,
            )
    nc.compile()
    return nc


_NC_CACHE = None


def kernel(p, a, W_ij):
    global _NC_CACHE
    from concourse.bass_utils import run_bass_kernel_spmd

    if _NC_CACHE is None:
        _NC_CACHE = _build()
    nc = _NC_CACHE

    import ml_dtypes
    p = np.asarray(p, dtype=np.float32).astype(ml_dtypes.bfloat16)
    Wf = np.asarray(W_ij, dtype=np.float32)          # (B, j, k, C)
    wsum = Wf.reshape(128, 128).astype(ml_dtypes.bfloat16)
    wbd = np.zeros((128, B * 128), dtype=np.float32)
    for b in range(B):
        wbd[b * P:(b + 1) * P, b * 128:(b + 1) * 128] = Wf[b].reshape(P, 128)
    wbd = wbd.astype(ml_dtypes.bfloat16)
    ident = np.eye(128, dtype=np.float32)
    in_maps = [
        {"p": p[2 * i:2 * i + 2], "Wbd": wbd, "Wsum": wsum, "ident": ident}
        for i in range(NCORES)
    ]
    res = run_bass_kernel_spmd(nc, in_maps, core_ids=list(range(NCORES)))
    v_parts = [res.results[i]["v_out"].reshape(BLOC, C, PP, H, WW)
               for i in range(NCORES)]
    a_parts = [res.results[i]["a_out"].reshape(BLOC, C, H, WW)
               for i in range(NCORES)]
    v_full = np.concatenate(v_parts, axis=0)
    a_full = np.concatenate(a_parts, axis=0)
    return v_full, a_full


# revision 43
# speedup vs baseline: 22.8666x; 22.8666x over previous
"""Capsule dynamic-routing kernel for 8 TRN2 NeuronCores.

Problem: nn_CapsuleRouting — p:(16,32,16,14,14), W_ij:(32,4,4,32), 3 routing
iterations, returns (v:(16,32,16,14,14), a:(16,32,14,14)).

Sharding: data-parallel over batch (2 batch elems per core, 8 cores).

Per-core design:
  - 392 positions (2 batch x 196 hw) in 4 chunks of 98 partitions.
  - SBUF layout: partitions = positions; u votes stored [98, B=32, ik=16, C=32]
    bf16.  Softmax coeffs c[B,C,pos] and squashed votes v[C,ik,pos] broadcast
    along free dims with step-0 APs (no replication needed).
  - Vote einsum on PE with a host-built block-diagonal W (K=128, all B per
    matmul); a plain stacked W contracts (B,j) to give iter0's s0 for free.
  - Reductions over B (weighted sum -> s) and over ik (r update) via PSUM
    accumulation with identity-lhsT matmuls on PE, pipelined behind the DVE
    bf16 2x multiply passes (w2 = c*u or u*v).
  - Small math (softmax exp, squash, reciprocal) on ACT/DVE per 2-chunk
    group so groups pipeline against each other's big passes.
"""

import numpy as np
from contextlib import ExitStack

P = 4
PP = 16
B = 32
C = 32
H = 14
WW = 14
POS = H * WW          # 196
HALF = POS // 2       # 98
BLOC = 2              # batch elems per core
NCH = 4               # chunks per core: (batch, half)
GRP = 1               # chunks per small-op group
ITERS = 3
EPS = 1e-5
NCORES = 8


def _build(debug=False, reps=1):
    import concourse.bass as bass
    import concourse.bacc as bacc
    import concourse.tile as tile
    from concourse import mybir

    f32 = mybir.dt.float32
    bf16 = mybir.dt.bfloat16
    AX = mybir.AxisListType
    AF = mybir.ActivationFunctionType

    nc = bacc.Bacc()
    p_ext = nc.declare_dram_parameter("p", [BLOC, B, PP, H, WW], bf16, isOutput=False)
    wbd_ext = nc.declare_dram_parameter("Wbd", [128, B * 128], bf16, isOutput=False)
    wsum_ext = nc.declare_dram_parameter("Wsum", [128, 128], bf16, isOutput=False)
    id_ext = nc.declare_dram_parameter("ident", [128, 128], f32, isOutput=False)
    v_ext = nc.declare_dram_parameter("v_out", [BLOC, C, PP, POS], f32, isOutput=True)
    a_ext = nc.declare_dram_parameter("a_out", [BLOC, C, POS], f32, isOutput=True)
    if debug:
        du_ext = nc.declare_dram_parameter(
            "dbg_u", [NCH, 98, B, PP, C], bf16, isOutput=True)
        ds_ext = nc.declare_dram_parameter(
            "dbg_s", [98, NCH, PP, C], f32, isOutput=True)

    def bcast(sl, axis, count):
        """Insert a step-0 (broadcast) dim into AP `sl` at free-dim position
        `axis` (0 = first free dim)."""
        ap = list(sl.ap)
        ap.insert(1 + axis, [0, count])
        return bass.AP(tensor=sl.tensor, offset=sl.offset, ap=ap)

    with tile.TileContext(nc) as tc, ExitStack() as ctx:
        singles = ctx.enter_context(tc.tile_pool(name="singles", bufs=1))
        upool = ctx.enter_context(tc.tile_pool(name="upool", bufs=4))
        w2pool = ctx.enter_context(tc.tile_pool(name="w2pool", bufs=3))
        small = ctx.enter_context(tc.tile_pool(name="small", bufs=1))
        outp = ctx.enter_context(tc.tile_pool(name="outp", bufs=2))
        mmps = ctx.enter_context(tc.tile_pool(name="mmps", bufs=2, space="PSUM"))
        sps = ctx.enter_context(tc.tile_pool(name="sps", bufs=2, space="PSUM"))
        tps = ctx.enter_context(tc.tile_pool(name="tps", bufs=2, space="PSUM"))

        # --- constants ---
        id_f = singles.tile([128, 128], f32)
        nc.sync.dma_start(out=id_f, in_=id_ext[:, :])
        id_b = singles.tile([128, 128], bf16)
        nc.scalar.copy(id_b, id_f)
        idb98 = id_b[:98, :98]
        idf98 = id_f[:98, :98]
        eps_t = singles.tile([98, 1], f32)
        nc.vector.memset(eps_t, EPS)
        wsum = singles.tile([128, 128], bf16)
        nc.sync.dma_start(out=wsum, in_=wsum_ext[:, :])

        # --- persistent state (chunk index is a free dim; ops slice groups) ---
        u = [upool.tile([98, B, PP, C], bf16, tag="u", name=f"u{i}")
             for i in range(NCH)]
        r_all = small.tile([98, NCH, B, C], bf16)      # routing logits
        c_all = small.tile([98, NCH, B, C], bf16)      # softmax coeffs
        s_all = small.tile([98, NCH, PP, C], f32)      # s, then v (in-place)
        vbf_all = small.tile([98, NCH, PP, C], bf16)   # v in bf16
        n2_all = small.tile([98, NCH, C], f32)
        nrm_all = small.tile([98, NCH, C], f32)
        sc_all = small.tile([98, NCH, C], f32)
        d_all = small.tile([98, NCH, B], f32)

        def squash(it, g2):
            """s -> v in place for chunk group g2; sets n2/nrm/sc slices."""
            c0 = g2 * GRP
            ssl = s_all[:, c0:c0 + GRP, :, :]
            # scratch for squares: alias the dead B<16 half of c_all (current
            # iter's c is already consumed; next softmax rewrites it fully)
            sq = c_all[:, c0:c0 + GRP, :PP, :]
            nc.gpsimd.tensor_mul(sq, ssl, ssl)
            n2 = n2_all[:, c0:c0 + GRP, :]
            nc.vector.tensor_reduce(
                n2, sq.rearrange("q n i c -> q n c i"),
                axis=AX.X, op=mybir.AluOpType.add,
            )
            nrm = nrm_all[:, c0:c0 + GRP, :]
            sc = sc_all[:, c0:c0 + GRP, :]
            nc.scalar.activation(nrm, n2, AF.Sqrt, bias=eps_t)
            nc.vector.tensor_scalar_add(sc, n2, 1.0)
            nc.vector.tensor_mul(sc, sc, nrm)          # (1+n2)*nrm
            nc.vector.reciprocal(sc, sc)
            nc.vector.tensor_mul(sc, sc, n2)           # n2/((1+n2)nrm)
            nc.vector.tensor_mul(ssl, ssl, bcast(sc, 1, PP))
            if it != ITERS - 1:
                nc.scalar.copy(vbf_all[:, c0:c0 + GRP, :, :], ssl)

        def softmax(g2):
            c0 = g2 * GRP
            rsl = r_all[:, c0:c0 + GRP, :, :]
            csl = c_all[:, c0:c0 + GRP, :, :]
            d = d_all[:, c0:c0 + GRP, :]
            nc.scalar.activation(csl, rsl, AF.Exp)
            nc.vector.tensor_reduce(d, csl, axis=AX.X, op=mybir.AluOpType.add)
            nc.vector.reciprocal(d, d)
            nc.gpsimd.tensor_mul(csl, csl, bcast(d, 2, C))

        def emit_outputs(g2):
            c0 = g2 * GRP
            asl = nrm_all[:, c0:c0 + GRP, :]
            # a = sqrt((n2/(1+n2))^2 + eps);  n2/(1+n2) = sc*nrm
            nc.vector.tensor_mul(asl, sc_all[:, c0:c0 + GRP, :], asl)
            nc.vector.tensor_mul(asl, asl, asl)
            nc.scalar.activation(asl, asl, AF.Sqrt, bias=eps_t)
            for ch in range(c0, c0 + GRP):
                bi, hf = divmod(ch, 2)
                vdst = v_ext[bi].rearrange("c (i k) s -> k c i s", i=P, k=P)
                for q in range(P):
                    tp = tps.tile([128, 98], f32, tag="tp")
                    nc.tensor.transpose(
                        tp,
                        s_all[:, ch, q * P:(q + 1) * P, :]
                        .rearrange("q a b -> q (a b)"),
                        idf98,
                    )
                    vt = outp.tile([128, 98], f32, tag="vt")
                    nc.scalar.copy(vt, tp)
                    nc.sync.dma_start(
                        out=vdst[:, :, q, hf * HALF:(hf + 1) * HALF],
                        in_=vt,
                    )
                tp = tps.tile([128, 98], f32, tag="tp")
                nc.tensor.transpose(tp[:C, :], nrm_all[:, ch, :], idf98)
                at = outp.tile([128, 98], f32, tag="vt")
                nc.scalar.copy(at[:C, :], tp[:C, :])
                nc.sync.dma_start(
                    out=a_ext[bi, :, hf * HALF:(hf + 1) * HALF],
                    in_=at[:C, :],
                )

        for _rep in range(reps):
            # --- phase 1: vote einsum ---
            if True:
                wbd = w2pool.tile([128, B * 128], bf16, tag="w2",
                                  name=f"wbd{_rep}")
                nc.sync.dma_start(out=wbd, in_=wbd_ext[:, :])
                for ch in range(NCH):
                    bi, hf = divmod(ch, 2)
                    src = p_ext[bi].rearrange(
                        "b (i j) h w -> b j i (h w)", i=P, j=P)
                    p_b = w2pool.tile([128, P, HALF], bf16, tag="w2",
                                      name=f"pb{_rep}_{ch}")
                    for i in range(P):
                        nc.sync.dma_start(
                            out=p_b[:, i, :],
                            in_=src[:, :, i, hf * HALF:(hf + 1) * HALF],
                        )
                    s0 = mmps.tile([98, P, 128], f32, tag="mm")
                    for i in range(P):
                        nc.tensor.matmul(s0[:, i, :], p_b[:, i, :], wsum,
                                         start=True, stop=True)
                    nc.scalar.activation(
                        s_all[:, ch, :, :], s0.rearrange("q i n -> q (i n)"),
                        AF.Copy, scale=1.0 / B,
                    )
                    for i in range(P):
                        for hp in range(4):
                            ps = mmps.tile([98, 8, P, C], f32, tag="mm")
                            for t in range(2):
                                pc = hp * 2 + t
                                nc.tensor.matmul(
                                    ps[:, t * 4:(t + 1) * 4, :, :],
                                    p_b[:, i, :],
                                    wbd[:, pc * 512:(pc + 1) * 512],
                                    start=True, stop=True,
                                )
                            dst = u[ch][:, hp * 8:(hp + 1) * 8,
                                        i * P:(i + 1) * P, :]
                            if hp == 0:
                                nc.vector.tensor_copy(dst, ps)
                            else:
                                nc.scalar.copy(dst, ps)

            # --- routing iterations (iter0 s came from the einsum) ---
            for it in range(ITERS):
                for g2 in range(NCH // GRP):
                    chs = range(g2 * GRP, (g2 + 1) * GRP)
                    if it > 0:
                        for ch in chs:
                            sp = sps.tile([98, PP, C], f32, tag="sp")
                            for sb in range(2):       # 16 B-caps per sub
                                w2 = w2pool.tile([98, 16, PP, C], bf16,
                                                 tag="w2", name=f"w2s{ch}{sb}")
                                csl = c_all[:, ch, sb * 16:(sb + 1) * 16, :]
                                nc.vector.tensor_mul(
                                    w2, u[ch][:, sb * 16:(sb + 1) * 16, :, :],
                                    bcast(csl, 1, PP),
                                )
                                for b16 in range(16):
                                    nc.tensor.matmul(
                                        sp[:, :, :], idb98, w2[:, b16, :, :],
                                        start=(sb == 0 and b16 == 0),
                                        stop=(sb == 1 and b16 == 15),
                                    )
                            nc.scalar.copy(s_all[:, ch, :, :], sp)
                    squash(it, g2)
                    if it == ITERS - 1:
                        emit_outputs(g2)
                        continue
                    for ch in chs:
                        dp = mmps.tile([98, B, C], f32, tag="mm",
                                       name=f"dp{it}_{ch}")
                        for sb in range(2):
                            w2 = w2pool.tile([98, 16, PP, C], bf16,
                                             tag="w2", name=f"w2d{ch}{sb}")
                            vsl = vbf_all[:, ch, :, :]
                            nc.vector.tensor_mul(
                                w2, u[ch][:, sb * 16:(sb + 1) * 16, :, :],
                                bcast(vsl, 0, 16),
                            )
                            for ik in range(PP):
                                nc.tensor.matmul(
                                    dp[:, sb * 16:(sb + 1) * 16, :],
                                    idb98, w2[:, :, ik, :],
                                    start=(ik == 0), stop=(ik == PP - 1),
                                )
                        if it == 0:
                            nc.scalar.copy(r_all[:, ch, :, :], dp)
                        else:
                            nc.vector.tensor_add(
                                r_all[:, ch, :, :], r_all[:, ch, :, :], dp)
                    softmax(g2)


            if debug:
                for ch in range(NCH):
                    nc.sync.dma_start(out=du_ext[ch], in_=u[ch])
                nc.sync.dma_start(out=ds_ext[:, :, :, :], in_=s_all)

    nc.compile()
    return nc


_NC_CACHE = None


def kernel(p, a, W_ij):
    global _NC_CACHE
    from concourse.bass_utils import run_bass_kernel_spmd

    if _NC_CACHE is None:
        _NC_CACHE = _build()
    nc = _NC_CACHE

    import ml_dtypes
    p = np.asarray(p, dtype=np.float32).astype(ml_dtypes.bfloat16)
    Wf = np.asarray(W_ij, dtype=np.float32)          # (B, j, k, C)
    wsum = Wf.reshape(128, 128).astype(ml_dtypes.bfloat16)
    wbd = np.zeros((128, B * 128), dtype=np.float32)
    for b in range(B):
        wbd[b * P:(b + 1) * P, b * 128:(b + 1) * 128] = Wf[b].reshape(P, 128)
    wbd = wbd.astype(ml_dtypes.bfloat16)
    ident = np.eye(128, dtype=np.float32)
    in_maps = [
        {"p": p[2 * i:2 * i + 2], "Wbd": wbd, "Wsum": wsum, "ident": ident}
        for i in range(NCORES)
    ]
    res = run_bass_kernel_spmd(nc, in_maps, core_ids=list(range(NCORES)))
    v_parts = [res.results[i]["v_out"].reshape(BLOC, C, PP, H, WW)
               for i in range(NCORES)]
    a_parts = [res.results[i]["a_out"].reshape(BLOC, C, H, WW)
               for i in range(NCORES)]
    v_full = np.concatenate(v_parts, axis=0)
    a_full = np.concatenate(a_parts, axis=0)
    return v_full, a_full
